# revision 34
# baseline (speedup 1.0000x reference)
"""Trainium2 Bass kernel for nn_CHSHistoryCrossAttentionFusion (8 NeuronCores, SPMD).

Decomposition (hardcoded for B=2, S=4096, L=3, D=1024, N=512, 8 cores):
  - History sequence-sharded: core c owns key positions [c*512, (c+1)*512) of
    each batch; it computes its chunk of fused/K/V from its x chunk.
  - Queries sharded 8-way for the Q path (see _core_gidx); an AllGather
    replicates Q (bf16, small) so every core scores all 1024 queries against
    its own K/V chunk.
  - Flash-style partial softmax per chunk WITHOUT max subtraction (Q/K are
    RMS-normalized so scores are bounded); causal mask applied additively
    before exp; exp carries a constant -ln(256) prescale so the (o,l)
    partials fit fp16.  Partials combine via fp16 ReduceScatter-adds:
    one RS for batch 0 (fully overlapped with batch-1 work) and TWO
    half-size RS for batch 1 so only the last 256-query chunk's wire time
    is exposed at the tail.
  - x is supplied pre-transposed by the host (layout-only change) so the
    fc contraction consumes it directly as the stationary operand — no
    PE transposes on the input side of phase 1.
  - Phase-1 matmuls (fc + Q-projection) run with BOTH operands typed
    float32r: the PE streams f32r at full (1 elem/cycle) moving rate for
    N>=512, so x/W_fc/Wq need NO on-chip f32->bf16 casts at all — the
    entire stage+cast+ring latency chain of the front end is gone and the
    AllGather fires ~60us earlier.  Phase-2 matmuls stay bf16 (fp32 acc)
    with weights staged+half-cast (scalar+vector in parallel) under
    relaxed deadlines.
  - DMA queues: weights and x slabs alternate across the scalar and sync
    HWDGE queues; x rides as [128, 2x512] kk-pair slabs with 2KB
    contiguous rows (fast descriptors).
Host-side work is layout/indexing only.
"""

import math
import os

import numpy as np

try:
    import ml_dtypes
except ImportError:  # pragma: no cover
    ml_dtypes = None

import concourse.bacc as bacc
import concourse.mybir as mybir
import concourse.tile as tile
import concourse.tile_utils as tile_utils
from concourse.bass_utils import run_bass_kernel_spmd

# cayman has 208 KiB/partition usable; the default constant leaves 16 KiB idle
tile_utils.max_sbuf_usage = 208 * 1024

F32 = mybir.dt.float32
F16 = mybir.dt.float16
BF16 = mybir.dt.bfloat16
AF = mybir.ActivationFunctionType
OP = mybir.AluOpType

B, S, L, D = 2, 4096, 3, 1024
N = 512
NC = 8
CH = S // NC              # 512 keys per batch per core
LD = L * D                # 3072
QT = B * N                # 1024 global queries
QPC = QT // NC            # 128 queries per core (64 b0 + 2x32 b1)
NKK = LD // 128           # 24 contraction slices over 3072
NJ = D // 128             # 8 contraction slices over 1024
RMS_EPS = 1e-6
SCALE = D ** -0.5
MASK_NEG = -60000.0
EXP_BIAS = -math.log(256.0)

_CACHE = {}


def _build(apply_norm_weights: bool):
    nc = bacc.Bacc("TRN2", target_bir_lowering=False, num_devices=NC)

    # ---------------- I/O ----------------
    F32R = mybir.dt.float32r
    # x transposed on host: [LD, B*CH] (cols 0:512 batch0, 512:1024 batch1).
    # Typed float32r end-to-end (raw f32 bits): PE streams f32r at full rate
    # as the moving operand (N>=256) and the verifier requires the producer
    # chain to be f32r-typed.
    xT = nc.dram_tensor("xT", [LD, B * CH], F32R, kind="ExternalInput")
    xqT = nc.dram_tensor("xqT", [LD, QPC], F32R, kind="ExternalInput")
    wfc = nc.dram_tensor("wfc", [LD, D], F32R, kind="ExternalInput")
    wq = nc.dram_tensor("wq", [D, D], F32R, kind="ExternalInput")
    wk = nc.dram_tensor("wk", [D, D], F32, kind="ExternalInput")
    wv = nc.dram_tensor("wv", [D, D], F32, kind="ExternalInput")
    wo = nc.dram_tensor("wo", [D, D], F32, kind="ExternalInput")
    identf = nc.dram_tensor("identf", [128, 128], F32R, kind="ExternalInput")
    # positional tables are host-synthesized constants -> ship as bf16
    pet = nc.dram_tensor("pet", [D, CH], BF16, kind="ExternalInput")
    peq = nc.dram_tensor("peq", [QPC, D], BF16, kind="ExternalInput")
    thr = nc.dram_tensor("thr", [128, NC], F32, kind="ExternalInput")
    iota = nc.dram_tensor("iota", [128, CH], F32, kind="ExternalInput")
    ident = nc.dram_tensor("ident", [128, 128], BF16, kind="ExternalInput")
    if apply_norm_weights:
        whn = nc.dram_tensor("whn", [128, D], F32, kind="ExternalInput")
        wqn = nc.dram_tensor("wqn", [128, D], F32, kind="ExternalInput")
        wkn = nc.dram_tensor("wkn", [128, D], F32, kind="ExternalInput")
        won = nc.dram_tensor("won", [128, D], F32, kind="ExternalInput")
    out = nc.dram_tensor("out", [QPC, D], F32, kind="ExternalOutput")

    # DRAM-side transposed views for the 3D gather loads
    xT_v = xT.ap().rearrange("(k p) n -> p k n", p=128)      # [128, 24, 1024]
    xqT_v = xqT.ap().rearrange("(k p) n -> p k n", p=128)    # [128, 24, 128]

    with tile.TileContext(nc) as tc:
        with (
            tc.tile_pool(name="dram", bufs=1, space="DRAM") as dram,
            tc.tile_pool(name="const", bufs=1) as constp,
            tc.tile_pool(name="stat", bufs=6) as stat,
            tc.tile_pool(name="base", bufs=1) as base,
            tc.tile_pool(name="scr_bf", bufs=3) as scr_bf,
            tc.tile_pool(name="scr_f", bufs=2) as scr_f,
            tc.tile_pool(name="mmps", bufs=2, space="PSUM") as mmps,
            tc.tile_pool(name="trps", bufs=2, space="PSUM") as trps,
            tc.tile_pool(name="scps", bufs=2, space="PSUM") as scps,
        ):
            # collective bounce buffers
            ag_in = dram.tile([QPC, D], BF16)
            ag_out = dram.tile([QT, D], BF16, addr_space="Shared")
            rs_inA = dram.tile([N, D + 1], F16)
            rs_outA = dram.tile([N // NC, D + 1], F16)
            rs_inB1 = dram.tile([N // 2, D + 1], F16)
            rs_outB1 = dram.tile([N // (2 * NC), D + 1], F16)
            rs_inB2 = dram.tile([N // 2, D + 1], F16)
            rs_outB2 = dram.tile([N // (2 * NC), D + 1], F16)

            # constants (bulk queue)
            id_sb = constp.tile([128, 128], BF16)
            nc.scalar.dma_start(id_sb[:], ident[:])
            idf_sb = constp.tile([128, 128], F32R)
            nc.scalar.dma_start(idf_sb[:], identf[:])
            iota_sb = constp.tile([128, CH], F32)
            nc.scalar.dma_start(iota_sb[:], iota[:])
            thr_sb = constp.tile([128, NC], F32)
            nc.scalar.dma_start(thr_sb[:], thr[:])
            eps_sb = constp.tile([128, 1], F32)
            nc.vector.memset(eps_sb[:], RMS_EPS)
            ebias_sb = constp.tile([128, 1], F32)
            nc.vector.memset(ebias_sb[:], EXP_BIAS)
            if apply_norm_weights:
                whn_sb = constp.tile([128, D], F32)
                nc.scalar.dma_start(whn_sb[:], whn[:])
                wqn_sb = constp.tile([128, D], F32)
                nc.scalar.dma_start(wqn_sb[:], wqn[:])
                wkn_sb = constp.tile([128, D], F32)
                nc.scalar.dma_start(wkn_sb[:], wkn[:])
                won_sb = constp.tile([128, D], F32)
                nc.scalar.dma_start(won_sb[:], won[:])

            # persistent activations (per-batch splits for fine-grained deps)
            fusedT_b = [base.tile([128, NJ * (4 * 128)], BF16, name=f"fusedT{b}")
                        for b in range(B)]
            fusedT_bv = [fT[:].rearrange("p (j t) -> p j t", j=NJ)
                         for fT in fusedT_b]
            qs_f32 = base.tile([QPC, D], F32)

            def rms_stats(src_ap):
                sq = scr_f.tile([128, D], F32, tag="sqscr")
                ssq = stat.tile([128, 1], F32, tag="ssq")
                nc.scalar.activation(sq[:], src_ap, AF.Square, accum_out=ssq[:])
                std = stat.tile([128, 1], F32, tag="std")
                nc.scalar.activation(std[:], ssq[:], AF.Sqrt, scale=1.0 / D,
                                     bias=eps_sb[:])
                rstd = stat.tile([128, 1], F32, tag="rstd")
                nc.vector.reciprocal(rstd[:], std[:])
                return rstd

            def transpose_to(dst_ap_3d, src_tile_ap, jlist):
                """PE-transpose 128x128 blocks into dst 3d view [128,len,128]."""
                ps = trps.tile([128, 512], BF16, tag="trp")
                for u, j in enumerate(jlist):
                    nc.tensor.transpose(
                        ps[:, u * 128:(u + 1) * 128],
                        src_tile_ap[:, j * 128:(j + 1) * 128],
                        id_sb[:],
                    )
                nc.vector.tensor_copy(
                    dst_ap_3d,
                    ps[:].rearrange("p (u x) -> p u x", u=len(jlist)),
                )

            # ---------------- phase 1: fc matmul for 9 token tiles ----------
            with (
                tc.tile_pool(name="ph1w", bufs=1) as ph1w,
                tc.tile_pool(name="ph1x", bufs=2) as ph1x,
            ):
                # Queue plan (FIFO per HWDGE queue, ~210 GB/s each).  x tiles
                # are interleaved with the weight loads on BOTH queues in
                # consumption order so the PE starts fc matmuls ~15us in and
                # never starves:
                #   sync  : xqT, x-q0, wfc[12:24], x-q2, wk[0:4], x-q4, x-q6,
                #           qT gather, RS payloads
                #   scalar: peq, wfc[0:12], x-q1, wq, x-q3, wk[4:8], pet,
                #           x-q5, x-q7, wv, wo
                # Cast engines: scalar takes wfc-A + odd x tiles; vector takes
                # wfc-B, wq, wk + even x tiles.  pet/peq ship as bf16.

                # sync queue head: the query x slab (512B descriptors --
                # slow-ish, but it's first and short); consumed directly as
                # the f32r stationary operand, no cast
                xqf = ph1x.tile([128, NKK * 128], F32R, tag="xqf", bufs=1)
                nc.sync.dma_start(
                    xqf[:].rearrange("p (k n) -> p k n", k=NKK), xqT_v)
                xq_r = xqf[:]
                # scalar queue head: peq (bf16, direct)
                peq_bf = ph1w.tile([QPC, D], BF16)
                nc.scalar.dma_start(peq_bf[:], peq[:])

                # W_fc: direct-resident float32r tiles, no staging/casts --
                # ready at pure queue rate, alternating queues per slice
                wfc_l = []
                for s_ in range(NKK):
                    wt = ph1w.tile([128, D], F32R, tag="wfc", bufs=NKK,
                                   name=f"wfc{s_}")
                    eng = nc.scalar if s_ % 2 else nc.sync
                    eng.dma_start(wt[:], wfc.ap()[s_ * 128:(s_ + 1) * 128, :])
                    wfc_l.append(wt)

                # Wq: 4-deep float32r ring (transient; consumed once by qps)
                wq_l = []
                for s_ in range(NJ):
                    wt = ph1w.tile([128, D], F32R, tag="wq", bufs=2,
                                   name=f"wq{s_}")
                    nc.scalar.dma_start(
                        wt[:], wq.ap()[s_ * 128:(s_ + 1) * 128, :])
                    wq_l.append(wt)

                # history x as kk-pair SLABS spanning all tokens of both
                # batches' chunks: [128, 2, 512] f32 slices of xT with 2KB
                # contiguous rows (fast descriptors).  12 slab-pairs per
                # batch resident as bf16; batch-1 slabs ring-reuse batch-0
                # slots once fc b0 has consumed them.
                NP = NKK // 2            # 12 kk-pairs
                xsl_b = [[ph1x.tile([128, 2 * CH], F32R, tag="xsl",
                                    bufs=NP, name=f"xsl{bb}_{pr}")
                          for pr in range(NP)] for bb in range(2)]

                def slab_load(bb, pr):
                    eng = nc.sync if pr % 2 == 0 else nc.scalar
                    eng.dma_start(
                        xsl_b[bb][pr][:].rearrange("p (k n) -> p k n", k=2),
                        xT_v[:, 2 * pr:2 * pr + 2, bb * CH:(bb + 1) * CH])

                def fc_lhsT(bb, pr_kk, tl):
                    pr, sub = divmod(pr_kk, 2)
                    off = sub * CH + tl * 128
                    return xsl_b[bb][pr][:, off:off + 128]

                for pr in range(NP):
                    slab_load(0, pr)

                # ---- query tile first (early AG) ----
                fps = mmps.tile([128, D], F32, tag="mm")
                for h in range(2):
                    for kk in range(NKK):
                        nc.tensor.matmul(
                            fps[:, h * 512:(h + 1) * 512],
                            xq_r[:, kk * 128:(kk + 1) * 128],
                            wfc_l[kk][:, h * 512:(h + 1) * 512],
                            start=(kk == 0),
                            stop=(kk == NKK - 1),
                        )
                rstd = rms_stats(fps[:])
                fb = scr_bf.tile([128, D], BF16, tag="tmb")
                nc.vector.tensor_scalar(fb[:], fps[:], rstd[:], None, OP.mult)
                if apply_norm_weights:
                    nc.vector.tensor_tensor(fb[:], fb[:], whn_sb[:],
                                            op=OP.mult)
                nc.vector.tensor_scalar(qs_f32[:], fps[:], rstd[:],
                                        None, OP.mult)
                if apply_norm_weights:
                    nc.vector.tensor_tensor(qs_f32[:], qs_f32[:],
                                            whn_sb[:], op=OP.mult)
                qhb = scr_f.tile([128, D], F32R, tag="sqscr")
                nc.vector.tensor_add(qhb[:], qs_f32[:], peq_bf[:])
                qht = scr_f.tile([128, D], F32R, tag="sqscr")
                for g in range(2):
                    psf = trps.tile([128, 512], F32R, tag="trp")
                    for u in range(4):
                        j = g * 4 + u
                        nc.tensor.transpose(
                            psf[:, u * 128:(u + 1) * 128],
                            qhb[:, j * 128:(j + 1) * 128],
                            idf_sb[:],
                        )
                    nc.vector.tensor_copy(
                        qht[:, g * 512:(g + 1) * 512], psf[:])
                qps = mmps.tile([128, D], F32, tag="mm")
                for j in range(NJ):
                    for h in range(2):
                        nc.tensor.matmul(
                            qps[:, h * 512:(h + 1) * 512],
                            qht[:, j * 128:(j + 1) * 128],
                            wq_l[j][:, h * 512:(h + 1) * 512],
                            start=(j == 0),
                            stop=(j == NJ - 1),
                        )
                qrstd = rms_stats(qps[:])
                qb = scr_bf.tile([128, D], BF16, tag="tmb")
                nc.vector.tensor_scalar(qb[:], qps[:], qrstd[:], None,
                                        OP.mult)
                if apply_norm_weights:
                    nc.vector.tensor_tensor(qb[:], qb[:], wqn_sb[:],
                                            op=OP.mult)
                nc.sync.dma_start(ag_in[:], qb[:])
                nc.gpsimd.collective_compute(
                    "AllGather", OP.bypass,
                    replica_groups=[list(range(NC))],
                    ins=[ag_in.opt()],
                    outs=[ag_out.opt()],
                )

                # ---- history tiles (batch-1 slabs issued while batch-0
                # tiles compute; ring reuse paces them automatically) ----
                for t in range(8):
                    bb, tl = divmod(t, 4)
                    if t < 3:
                        for pr in range(4 * t, 4 * (t + 1)):
                            slab_load(1, pr)
                    fps = mmps.tile([128, D], F32, tag="mm")
                    for h in range(2):
                        for kk in range(NKK):
                            nc.tensor.matmul(
                                fps[:, h * 512:(h + 1) * 512],
                                fc_lhsT(bb, kk, tl),
                                wfc_l[kk][:, h * 512:(h + 1) * 512],
                                start=(kk == 0),
                                stop=(kk == NKK - 1),
                            )
                    rstd = rms_stats(fps[:])
                    fb = scr_bf.tile([128, D], BF16, tag="tmb")
                    nc.vector.tensor_scalar(fb[:], fps[:], rstd[:], None,
                                            OP.mult)
                    if apply_norm_weights:
                        nc.vector.tensor_tensor(fb[:], fb[:], whn_sb[:],
                                                op=OP.mult)
                    for g in range(2):
                        transpose_to(
                            fusedT_bv[bb][:, g * 4:(g + 1) * 4,
                                          tl * 128:(tl + 1) * 128],
                            fb[:],
                            [g * 4 + u for u in range(4)],
                        )

            # -------- phase 2: K^T, V per batch; qT; attention; RS ----------
            with (
                tc.tile_pool(name="ph2w", bufs=1) as ph2w,
            ):
                kT_b = [ph2w.tile([128, NJ * CH], BF16, name=f"kT{b}")
                        for b in range(B)]
                kT_bv = [kT[:].rearrange("p (j t) -> p j t", j=NJ)
                         for kT in kT_b]
                v_b = [ph2w.tile([128, 4 * D], BF16, name=f"v{b}")
                       for b in range(B)]
                qT = ph2w.tile([128, NJ * QT], BF16)
                qT_v = qT[:].rearrange("p (j t) -> p j t", j=NJ)

                def half_cast(dst, srct):
                    nc.scalar.copy(dst[:, 0:512], srct[:, 0:512])
                    nc.vector.tensor_copy(dst[:, 512:1024], srct[:, 512:1024])

                def load_w_slices(src, nm, tag=None):
                    tiles = []
                    for s_ in range(NJ):
                        wst = ph2w.tile([128, D], F32, tag="wst2", bufs=3,
                                        name=f"{nm}st{s_}")
                        eng = nc.sync if s_ % 2 == 0 else nc.scalar
                        eng.dma_start(
                            wst[:], src.ap()[s_ * 128:(s_ + 1) * 128, :])
                        wt = ph2w.tile([128, D], BF16, tag=(tag or nm),
                                       bufs=NJ, name=f"{nm}{s_}")
                        half_cast(wt, wst)
                        tiles.append(wt)
                    return tiles

                wk_l = load_w_slices(wk, "wk")
                pet_bf = ph2w.tile([128, NJ * CH], BF16)     # [d_lo, j*512+tc]
                nc.scalar.dma_start(
                    pet_bf[:].rearrange("p (j t) -> p j t", j=NJ),
                    pet.ap().rearrange("(j p) t -> p j t", p=128))
                pet_v = pet_bf[:].rearrange("p (j t) -> p j t", j=NJ)
                wv_l = load_w_slices(wv, "wv")

                def k_tile(bb, tl):
                    khb = ph2w.tile([128, NJ * 128], BF16, tag="khb", bufs=2)
                    nc.vector.tensor_add(
                        khb[:].rearrange("p (j x) -> p j x", j=NJ),
                        fusedT_bv[bb][:, :, tl * 128:(tl + 1) * 128],
                        pet_v[:, :, tl * 128:(tl + 1) * 128],
                    )
                    kps = mmps.tile([128, D], F32, tag="mm")
                    for h in range(2):
                        for j in range(NJ):
                            nc.tensor.matmul(
                                kps[:, h * 512:(h + 1) * 512],
                                khb[:, j * 128:(j + 1) * 128],
                                wk_l[j][:, h * 512:(h + 1) * 512],
                                start=(j == 0),
                                stop=(j == NJ - 1),
                            )
                    krstd = rms_stats(kps[:])
                    kb = scr_bf.tile([128, D], BF16, tag="tmb")
                    nc.vector.tensor_scalar(kb[:], kps[:], krstd[:], None,
                                            OP.mult)
                    if apply_norm_weights:
                        nc.vector.tensor_tensor(kb[:], kb[:], wkn_sb[:],
                                                op=OP.mult)
                    for g in range(2):
                        transpose_to(
                            kT_bv[bb][:, g * 4:(g + 1) * 4,
                                      tl * 128:(tl + 1) * 128],
                            kb[:],
                            [g * 4 + u for u in range(4)],
                        )

                def v_tile(bb, tl):
                    for h in range(2):
                        vps = scps.tile([128, 512], F32, tag="sc")
                        for j in range(NJ):
                            nc.tensor.matmul(
                                vps[:],
                                fusedT_bv[bb][:, j:j + 1,
                                              tl * 128:(tl + 1) * 128]
                                .rearrange("p j x -> p (j x)"),
                                wv_l[j][:, h * 512:(h + 1) * 512],
                                start=(j == 0),
                                stop=(j == NJ - 1),
                            )
                        nc.vector.tensor_copy(
                            v_b[bb][:, tl * D + h * 512: tl * D + h * 512 + 512],
                            vps[:])

                def attn_tile(i, rs_buf, row0, after=None):
                    bchunk = 0 if i < 4 else 1
                    sps = scps.tile([128, 512], F32, tag="sc")
                    first_mm = None
                    for j in range(NJ):
                        mm_i = nc.tensor.matmul(
                            sps[:],
                            qT[:, j * QT + i * 128: j * QT + (i + 1) * 128],
                            kT_b[bchunk][:, j * CH:(j + 1) * CH],
                            start=(j == 0),
                            stop=(j == NJ - 1),
                        )
                        if first_mm is None:
                            first_mm = mm_i
                            if after is not None:
                                tile.add_dep_helper(
                                    first_mm.ins,
                                    after.ins,
                                    sync=False,
                                    reason="order attn after prior RS inputs")
                    mb = ph2w.tile([128, CH], F16, tag="mb", bufs=2)
                    nc.vector.tensor_scalar(mb[:], iota_sb[:],
                                            thr_sb[:, i:i + 1], MASK_NEG,
                                            OP.is_gt, OP.mult)
                    sm = ph2w.tile([128, CH], F32, tag="sm", bufs=2)
                    nc.vector.tensor_add(sm[:], sps[:], mb[:])
                    o_sb = ph2w.tile([128, D + 1], F16, tag="osb", bufs=2)
                    lacc = stat.tile([128, 1], F32, tag="lacc")
                    probs = ph2w.tile([128, CH], BF16, tag="probs", bufs=2)
                    nc.scalar.activation(probs[:], sm[:], AF.Exp, scale=SCALE,
                                         bias=ebias_sb[:], accum_out=lacc[:])
                    nc.vector.tensor_copy(o_sb[:, D:D + 1], lacc[:])
                    pps = trps.tile([128, 512], BF16, tag="trp")
                    for u in range(4):
                        nc.tensor.transpose(
                            pps[:, u * 128:(u + 1) * 128],
                            probs[:, u * 128:(u + 1) * 128],
                            id_sb[:],
                        )
                    pT = ph2w.tile([128, 512], BF16, tag="pT", bufs=2)
                    nc.vector.tensor_copy(pT[:], pps[:])
                    ops_ = mmps.tile([128, D], F32, tag="mm")
                    for h in range(2):
                        for u in range(4):
                            nc.tensor.matmul(
                                ops_[:, h * 512:(h + 1) * 512],
                                pT[:, u * 128:(u + 1) * 128],
                                v_b[bchunk][:, u * D + h * 512:
                                            u * D + h * 512 + 512],
                                start=(u == 0),
                                stop=(u == 3),
                            )
                    nc.vector.tensor_copy(o_sb[:, 0:D], ops_[:])
                    return nc.sync.dma_start(rs_buf[row0:row0 + 128, :],
                                             o_sb[:])

                for tl in range(4):
                    k_tile(0, tl)
                    v_tile(0, tl)
                wo_l = load_w_slices(wo, "wo", tag="wk")

                # qT gather from AG output.
                # batch0 tiles (i<4): queries 128i+k owned 64-apiece by cores
                # 2i, 2i+1 (first 64 rows of their AG block).
                # batch1 tiles: 32-query blocks; tiles 4,6 from cores 0-3,
                # tiles 5,7 from cores 4-7; rows 64:96 (first half) or
                # 96:128 (second half) of the AG block.
                for i in range(NC):
                    qg = ph2w.tile([128, D], BF16, tag="qg", bufs=4)
                    if i < 4:
                        r0 = (2 * i) * 128
                        r1 = (2 * i + 1) * 128
                        nc.sync.dma_start(qg[0:64, :], ag_out[r0:r0 + 64, :])
                        nc.sync.dma_start(qg[64:128, :], ag_out[r1:r1 + 64, :])
                    else:
                        half = (i - 4) // 2          # 0 for tiles 4,5; 1 for 6,7
                        cbase = 4 * ((i - 4) % 2)    # cores 0-3 or 4-7
                        srow = 64 + 32 * half
                        for m in range(4):
                            r = (cbase + m) * 128 + srow
                            nc.sync.dma_start(
                                qg[32 * m:32 * m + 32, :],
                                ag_out[r:r + 32, :])
                    for g in range(2):
                        transpose_to(
                            qT_v[:, g * 4:(g + 1) * 4, i * 128:(i + 1) * 128],
                            qg[:],
                            [g * 4 + u for u in range(4)],
                        )

                last_dma = None
                for i in range(4):
                    last_dma = attn_tile(i, rs_inA, i * 128)
                    k_tile(1, i)
                    v_tile(1, i)
                nc.gpsimd.collective_compute(
                    "ReduceScatter", OP.add,
                    replica_groups=[list(range(NC))],
                    ins=[rs_inA.opt()],
                    outs=[rs_outA.opt()],
                )
                for i in (4, 5):
                    last_dma = attn_tile(i, rs_inB1, (i - 4) * 128,
                                         after=last_dma)
                nc.gpsimd.collective_compute(
                    "ReduceScatter", OP.add,
                    replica_groups=[list(range(NC))],
                    ins=[rs_inB1.opt()],
                    outs=[rs_outB1.opt()],
                )
                for i in (6, 7):
                    last_dma = attn_tile(i, rs_inB2, (i - 6) * 128,
                                         after=last_dma)
                nc.gpsimd.collective_compute(
                    "ReduceScatter", OP.add,
                    replica_groups=[list(range(NC))],
                    ins=[rs_inB2.opt()],
                    outs=[rs_outB2.opt()],
                )

                # ---------------- epilogue for own 128 queries --------------
                fo = ph2w.tile([QPC, D + 1], F16, tag="fo", bufs=1)
                nc.sync.dma_start(fo[0:64, :], rs_outA[:])
                nc.sync.dma_start(fo[64:96, :], rs_outB1[:])
                nc.sync.dma_start(fo[96:128, :], rs_outB2[:])
                linv = stat.tile([128, 1], F32, tag="linv")
                nc.vector.reciprocal(linv[:], fo[:, D:D + 1])
                ao = scr_bf.tile([128, D], BF16, tag="tmb")
                nc.vector.tensor_scalar(ao[:], fo[:, 0:D], linv[:], None,
                                        OP.mult)
                aoT = scr_bf.tile([128, D], BF16, tag="tmb")
                aoT_v = aoT[:].rearrange("p (g x) -> p g x", g=2)
                for g in range(2):
                    transpose_to(
                        aoT_v[:, g:g + 1, :].rearrange("p g x -> p (g x)")
                        .rearrange("p (u x) -> p u x", u=4),
                        ao[:],
                        [g * 4 + u for u in range(4)],
                    )
                zps = mmps.tile([128, D], F32, tag="mm")
                for h in range(2):
                    for j in range(NJ):
                        nc.tensor.matmul(
                            zps[:, h * 512:(h + 1) * 512],
                            aoT[:, j * 128:(j + 1) * 128],
                            wo_l[j][:, h * 512:(h + 1) * 512],
                            start=(j == 0),
                            stop=(j == NJ - 1),
                        )
                hh = scr_f.tile([128, D], F32, tag="sqscr")
                nc.vector.tensor_add(hh[:], qs_f32[:], zps[:])
                orstd = rms_stats(hh[:])
                yv = scr_f.tile([128, D], F32, tag="sqscr")
                nc.vector.tensor_scalar(yv[:], hh[:], orstd[:], None, OP.mult)
                if apply_norm_weights:
                    nc.vector.tensor_tensor(yv[:], yv[:], won_sb[:],
                                            op=OP.mult)
                nc.sync.dma_start(out[:], yv[:])

    nc.compile()
    return nc


def _pe_table():
    half = D // 2
    inv_freq = np.exp(np.arange(half, dtype=np.float32)
                      * (-math.log(10000.0) / half))
    ang = np.arange(S, dtype=np.float32)[:, None] * inv_freq
    return np.concatenate([np.sin(ang), np.cos(ang)], axis=-1).astype(np.float32)


def _core_gidx(c):
    """Global query indices owned by core c: 64 batch-0 queries
    [64c, 64c+64), then batch-1 queries [32c, 32c+32) and
    [256+32c, 256+32c+32) — matching the RS_A / RS_B1 / RS_B2 splits."""
    return np.concatenate([
        np.arange(64 * c, 64 * c + 64),
        N + np.arange(32 * c, 32 * c + 32),
        N + 256 + np.arange(32 * c, 32 * c + 32),
    ])


def make_in_maps(np_inputs, apply_w=False):
    hid = np.asarray(np_inputs["hidden_states"], np.float32)
    pos = np.asarray(np_inputs["context_positions"])
    Wfc = np.ascontiguousarray(np.asarray(np_inputs["W_fc"], np.float32))
    Wq = np.ascontiguousarray(np.asarray(np_inputs["Wq"], np.float32))
    Wk = np.ascontiguousarray(np.asarray(np_inputs["Wk"], np.float32))
    Wv = np.ascontiguousarray(np.asarray(np_inputs["Wv"], np.float32))
    Wo = np.ascontiguousarray(np.asarray(np_inputs["Wo"], np.float32))

    x = hid.reshape(B, S, LD)
    p = np.clip(pos.astype(np.int64), 0, S - 1)
    p_flat = p.reshape(QT)
    PE = _pe_table()

    iota_np = np.tile(np.arange(CH, dtype=np.float32), (128, 1))
    ident_np = np.eye(128, dtype=np.float32).astype(ml_dtypes.bfloat16)
    identf_np = np.eye(128, dtype=np.float32)

    in_maps = []
    for c in range(NC):
        sl = slice(c * CH, (c + 1) * CH)
        xT_c = np.ascontiguousarray(
            np.concatenate([x[0, sl], x[1, sl]], axis=0).T)
        gidx = _core_gidx(c)
        xqT_a = np.ascontiguousarray(x[gidx // N, p_flat[gidx]].T)
        peq_a = np.ascontiguousarray(PE[p_flat[gidx]]).astype(ml_dtypes.bfloat16)
        pet_a = np.ascontiguousarray(PE[sl].T).astype(ml_dtypes.bfloat16)
        thr_a = np.ascontiguousarray(
            (p_flat.astype(np.float32) - c * CH).reshape(NC, 128).T)
        m = {
            "xT": xT_c, "xqT": xqT_a,
            "wfc": Wfc, "wq": Wq, "wk": Wk, "wv": Wv, "wo": Wo,
            "pet": pet_a, "peq": peq_a, "thr": thr_a,
            "iota": iota_np, "ident": ident_np, "identf": identf_np,
        }
        if apply_w:
            m["whn"] = np.tile(np.asarray(np_inputs["w_hidden_norm"], np.float32), (128, 1))
            m["wqn"] = np.tile(np.asarray(np_inputs["w_q_norm"], np.float32), (128, 1))
            m["wkn"] = np.tile(np.asarray(np_inputs["w_k_norm"], np.float32), (128, 1))
            m["won"] = np.tile(np.asarray(np_inputs["w_out_norm"], np.float32), (128, 1))
        in_maps.append(m)
    return in_maps


def assemble_out(results):
    y = np.zeros((QT, D), np.float32)
    for c in range(NC):
        y[_core_gidx(c)] = results[c]["out"]
    return y.reshape(B, N, D)


def kernel(**inputs) -> np.ndarray:
    w_h = np.asarray(inputs["w_hidden_norm"], np.float32)
    w_q = np.asarray(inputs["w_q_norm"], np.float32)
    w_k = np.asarray(inputs["w_k_norm"], np.float32)
    w_o = np.asarray(inputs["w_out_norm"], np.float32)
    apply_w = not (np.all(w_h == 1) and np.all(w_q == 1)
                   and np.all(w_k == 1) and np.all(w_o == 1))

    key = ("nc", apply_w)
    if key not in _CACHE:
        _CACHE[key] = _build(apply_w)
    nc = _CACHE[key]

    in_maps = make_in_maps(inputs, apply_w)

    trace = os.environ.get("KERNEL_TRACE", "0") == "1"
    if trace:
        try:
            import axon_prof
            axon_prof.install()
        except Exception:
            trace = False
    res = run_bass_kernel_spmd(nc, in_maps, list(range(NC)), trace=trace)
    global LAST_EXEC_NS
    LAST_EXEC_NS = res.exec_time_ns

    return assemble_out(res.results).astype(np.float32)


LAST_EXEC_NS = None


# revision 36
# speedup vs baseline: 1.0487x; 1.0487x over previous
"""Trainium2 Bass kernel for nn_CHSHistoryCrossAttentionFusion (8 NeuronCores, SPMD).

Decomposition (hardcoded for B=2, S=4096, L=3, D=1024, N=512, 8 cores):
  - History sequence-sharded: core c owns key positions [c*512, (c+1)*512) of
    each batch; it computes its chunk of fused/K/V from its x chunk.
  - Queries sharded 8-way for the Q path (see _core_gidx); an AllGather
    replicates Q (bf16, small) so every core scores all 1024 queries against
    its own K/V chunk.
  - Flash-style partial softmax per chunk WITHOUT max subtraction (Q/K are
    RMS-normalized so scores are bounded); causal mask applied additively
    before exp; exp carries a constant -ln(256) prescale so the (o,l)
    partials fit fp16.  Partials combine via fp16 ReduceScatter-adds:
    one RS for batch 0 (fully overlapped with batch-1 work) and TWO
    half-size RS for batch 1 so only the last 256-query chunk's wire time
    is exposed at the tail.
  - x is supplied pre-transposed by the host (layout-only change) so the
    fc contraction consumes it directly as the stationary operand — no
    PE transposes on the input side of phase 1.
  - Phase-1 matmuls (fc + Q-projection) run with BOTH operands typed
    float32r: the PE streams f32r at full (1 elem/cycle) moving rate for
    N>=512, so x/W_fc/Wq need NO on-chip f32->bf16 casts at all — the
    entire stage+cast+ring latency chain of the front end is gone and the
    AllGather fires ~60us earlier.  Phase-2 matmuls stay bf16 (fp32 acc)
    with weights staged+half-cast (scalar+vector in parallel) under
    relaxed deadlines.
  - DMA queues: weights and x slabs alternate across the scalar and sync
    HWDGE queues; x rides as [128, 2x512] kk-pair slabs with 2KB
    contiguous rows (fast descriptors).
Host-side work is layout/indexing only.
"""

import math
import os

import numpy as np

try:
    import ml_dtypes
except ImportError:  # pragma: no cover
    ml_dtypes = None

import concourse.bacc as bacc
import concourse.mybir as mybir
import concourse.tile as tile
import concourse.tile_utils as tile_utils
from concourse.bass_utils import run_bass_kernel_spmd

# cayman has 208 KiB/partition usable; the default constant leaves 16 KiB idle
tile_utils.max_sbuf_usage = 208 * 1024

F32 = mybir.dt.float32
F16 = mybir.dt.float16
BF16 = mybir.dt.bfloat16
AF = mybir.ActivationFunctionType
OP = mybir.AluOpType

B, S, L, D = 2, 4096, 3, 1024
N = 512
NC = 8
CH = S // NC              # 512 keys per batch per core
LD = L * D                # 3072
QT = B * N                # 1024 global queries
QPC = QT // NC            # 128 queries per core (64 b0 + 2x32 b1)
NKK = LD // 128           # 24 contraction slices over 3072
NJ = D // 128             # 8 contraction slices over 1024
RMS_EPS = 1e-6
SCALE = D ** -0.5
MASK_NEG = -60000.0
EXP_BIAS = -math.log(256.0)

_CACHE = {}


def _build(apply_norm_weights: bool):
    nc = bacc.Bacc("TRN2", target_bir_lowering=False, num_devices=NC)

    # ---------------- I/O ----------------
    F32R = mybir.dt.float32r
    # x transposed on host: [LD, B*CH] (cols 0:512 batch0, 512:1024 batch1).
    # Typed float32r end-to-end (raw f32 bits): PE streams f32r at full rate
    # as the moving operand (N>=256) and the verifier requires the producer
    # chain to be f32r-typed.
    xT = nc.dram_tensor("xT", [LD, B * CH], F32R, kind="ExternalInput")
    xqT = nc.dram_tensor("xqT", [LD, QPC], F32R, kind="ExternalInput")
    wfc = nc.dram_tensor("wfc", [LD, D], F32R, kind="ExternalInput")
    wq = nc.dram_tensor("wq", [D, D], F32R, kind="ExternalInput")
    wk = nc.dram_tensor("wk", [D, D], F32, kind="ExternalInput")
    wv = nc.dram_tensor("wv", [D, D], F32, kind="ExternalInput")
    wo = nc.dram_tensor("wo", [D, D], F32, kind="ExternalInput")
    identf = nc.dram_tensor("identf", [128, 128], F32R, kind="ExternalInput")
    # positional tables are host-synthesized constants -> ship as bf16
    pet = nc.dram_tensor("pet", [D, CH], BF16, kind="ExternalInput")
    peq = nc.dram_tensor("peq", [QPC, D], BF16, kind="ExternalInput")
    thr = nc.dram_tensor("thr", [128, NC], F32, kind="ExternalInput")
    iota = nc.dram_tensor("iota", [128, CH], F32, kind="ExternalInput")
    ident = nc.dram_tensor("ident", [128, 128], BF16, kind="ExternalInput")
    if apply_norm_weights:
        whn = nc.dram_tensor("whn", [128, D], F32, kind="ExternalInput")
        wqn = nc.dram_tensor("wqn", [128, D], F32, kind="ExternalInput")
        wkn = nc.dram_tensor("wkn", [128, D], F32, kind="ExternalInput")
        won = nc.dram_tensor("won", [128, D], F32, kind="ExternalInput")
    out = nc.dram_tensor("out", [QPC, D], F32, kind="ExternalOutput")

    # DRAM-side transposed views for the 3D gather loads
    xT_v = xT.ap().rearrange("(k p) n -> p k n", p=128)      # [128, 24, 1024]
    xqT_v = xqT.ap().rearrange("(k p) n -> p k n", p=128)    # [128, 24, 128]

    with tile.TileContext(nc) as tc:
        with (
            tc.tile_pool(name="dram", bufs=1, space="DRAM") as dram,
            tc.tile_pool(name="const", bufs=1) as constp,
            tc.tile_pool(name="stat", bufs=6) as stat,
            tc.tile_pool(name="base", bufs=1) as base,
            tc.tile_pool(name="scr_bf", bufs=3) as scr_bf,
            tc.tile_pool(name="scr_f", bufs=2) as scr_f,
            tc.tile_pool(name="mmps", bufs=2, space="PSUM") as mmps,
            tc.tile_pool(name="trps", bufs=2, space="PSUM") as trps,
            tc.tile_pool(name="scps", bufs=2, space="PSUM") as scps,
        ):
            # collective bounce buffers
            ag_in = dram.tile([QPC, D], BF16)
            ag_out = dram.tile([QT, D], BF16, addr_space="Shared")
            rs_inA = dram.tile([N, D + 1], F16)
            rs_outA = dram.tile([N // NC, D + 1], F16)
            rs_inB1 = dram.tile([N // 2, D + 1], F16)
            rs_outB1 = dram.tile([N // (2 * NC), D + 1], F16)
            rs_inB2 = dram.tile([N // 2, D + 1], F16)
            rs_outB2 = dram.tile([N // (2 * NC), D + 1], F16)

            # constants (bulk queue)
            id_sb = constp.tile([128, 128], BF16)
            nc.scalar.dma_start(id_sb[:], ident[:])
            idf_sb = constp.tile([128, 128], F32R)
            nc.scalar.dma_start(idf_sb[:], identf[:])
            iota_sb = constp.tile([128, CH], F32)
            nc.scalar.dma_start(iota_sb[:], iota[:])
            thr_sb = constp.tile([128, NC], F32)
            nc.scalar.dma_start(thr_sb[:], thr[:])
            eps_sb = constp.tile([128, 1], F32)
            nc.vector.memset(eps_sb[:], RMS_EPS)
            ebias_sb = constp.tile([128, 1], F32)
            nc.vector.memset(ebias_sb[:], EXP_BIAS)
            if apply_norm_weights:
                whn_sb = constp.tile([128, D], F32)
                nc.scalar.dma_start(whn_sb[:], whn[:])
                wqn_sb = constp.tile([128, D], F32)
                nc.scalar.dma_start(wqn_sb[:], wqn[:])
                wkn_sb = constp.tile([128, D], F32)
                nc.scalar.dma_start(wkn_sb[:], wkn[:])
                won_sb = constp.tile([128, D], F32)
                nc.scalar.dma_start(won_sb[:], won[:])

            # persistent activations (per-batch splits for fine-grained deps)
            fusedT_b = [base.tile([128, NJ * (4 * 128)], BF16, name=f"fusedT{b}")
                        for b in range(B)]
            fusedT_bv = [fT[:].rearrange("p (j t) -> p j t", j=NJ)
                         for fT in fusedT_b]
            qs_f32 = base.tile([QPC, D], F32)

            def rms_stats(src_ap):
                sq = scr_f.tile([128, D], F32, tag="sqscr")
                ssq = stat.tile([128, 1], F32, tag="ssq")
                nc.scalar.activation(sq[:], src_ap, AF.Square, accum_out=ssq[:])
                std = stat.tile([128, 1], F32, tag="std")
                nc.scalar.activation(std[:], ssq[:], AF.Sqrt, scale=1.0 / D,
                                     bias=eps_sb[:])
                rstd = stat.tile([128, 1], F32, tag="rstd")
                nc.vector.reciprocal(rstd[:], std[:])
                return rstd

            def transpose_to(dst_ap_3d, src_tile_ap, jlist):
                """PE-transpose 128x128 blocks into dst 3d view [128,len,128]."""
                ps = trps.tile([128, 512], BF16, tag="trp")
                for u, j in enumerate(jlist):
                    nc.tensor.transpose(
                        ps[:, u * 128:(u + 1) * 128],
                        src_tile_ap[:, j * 128:(j + 1) * 128],
                        id_sb[:],
                    )
                nc.vector.tensor_copy(
                    dst_ap_3d,
                    ps[:].rearrange("p (u x) -> p u x", u=len(jlist)),
                )

            # ---------------- phase 1: fc matmul for 9 token tiles ----------
            with (
                tc.tile_pool(name="ph1w", bufs=1) as ph1w,
                tc.tile_pool(name="ph1x", bufs=2) as ph1x,
            ):
                # Queue plan (FIFO per HWDGE queue, ~210 GB/s each).  x tiles
                # are interleaved with the weight loads on BOTH queues in
                # consumption order so the PE starts fc matmuls ~15us in and
                # never starves:
                #   sync  : xqT, x-q0, wfc[12:24], x-q2, wk[0:4], x-q4, x-q6,
                #           qT gather, RS payloads
                #   scalar: peq, wfc[0:12], x-q1, wq, x-q3, wk[4:8], pet,
                #           x-q5, x-q7, wv, wo
                # Cast engines: scalar takes wfc-A + odd x tiles; vector takes
                # wfc-B, wq, wk + even x tiles.  pet/peq ship as bf16.

                # sync queue head: the query x slab (512B descriptors --
                # slow-ish, but it's first and short); consumed directly as
                # the f32r stationary operand, no cast
                xqf = ph1x.tile([128, NKK * 128], F32R, tag="xqf", bufs=1)
                nc.sync.dma_start(
                    xqf[:].rearrange("p (k n) -> p k n", k=NKK), xqT_v)
                xq_r = xqf[:]
                # scalar queue head: peq (bf16, direct)
                peq_bf = ph1w.tile([QPC, D], BF16)
                nc.scalar.dma_start(peq_bf[:], peq[:])

                # W_fc: direct-resident float32r tiles, no staging/casts --
                # ready at pure queue rate, alternating queues per slice
                wfc_l = []
                for s_ in range(NKK):
                    wt = ph1w.tile([128, D], F32R, tag="wfc", bufs=NKK,
                                   name=f"wfc{s_}")
                    eng = nc.scalar if s_ % 2 else nc.sync
                    eng.dma_start(wt[:], wfc.ap()[s_ * 128:(s_ + 1) * 128, :])
                    wfc_l.append(wt)

                # Wq: 4-deep float32r ring (transient; consumed once by qps)
                wq_l = []
                for s_ in range(NJ):
                    wt = ph1w.tile([128, D], F32R, tag="wq", bufs=2,
                                   name=f"wq{s_}")
                    nc.scalar.dma_start(
                        wt[:], wq.ap()[s_ * 128:(s_ + 1) * 128, :])
                    wq_l.append(wt)

                # history x as kk-pair SLABS spanning all tokens of both
                # batches' chunks: [128, 2, 512] f32 slices of xT with 2KB
                # contiguous rows (fast descriptors).  12 slab-pairs per
                # batch resident as bf16; batch-1 slabs ring-reuse batch-0
                # slots once fc b0 has consumed them.
                NP = NKK // 2            # 12 kk-pairs
                xsl_b = [[ph1x.tile([128, 2 * CH], F32R, tag="xsl",
                                    bufs=NP, name=f"xsl{bb}_{pr}")
                          for pr in range(NP)] for bb in range(2)]

                def slab_load(bb, pr):
                    eng = nc.sync if pr % 2 == 0 else nc.scalar
                    eng.dma_start(
                        xsl_b[bb][pr][:].rearrange("p (k n) -> p k n", k=2),
                        xT_v[:, 2 * pr:2 * pr + 2, bb * CH:(bb + 1) * CH])

                def fc_lhsT(bb, pr_kk, tl):
                    pr, sub = divmod(pr_kk, 2)
                    off = sub * CH + tl * 128
                    return xsl_b[bb][pr][:, off:off + 128]

                for pr in range(NP):
                    slab_load(0, pr)

                # ---- query tile first (early AG) ----
                fps = mmps.tile([128, D], F32, tag="mm")
                for h in range(2):
                    for kk in range(NKK):
                        nc.tensor.matmul(
                            fps[:, h * 512:(h + 1) * 512],
                            xq_r[:, kk * 128:(kk + 1) * 128],
                            wfc_l[kk][:, h * 512:(h + 1) * 512],
                            start=(kk == 0),
                            stop=(kk == NKK - 1),
                        )
                rstd = rms_stats(fps[:])
                fb = scr_bf.tile([128, D], BF16, tag="tmb")
                nc.vector.tensor_scalar(fb[:], fps[:], rstd[:], None, OP.mult)
                if apply_norm_weights:
                    nc.vector.tensor_tensor(fb[:], fb[:], whn_sb[:],
                                            op=OP.mult)
                nc.vector.tensor_scalar(qs_f32[:], fps[:], rstd[:],
                                        None, OP.mult)
                if apply_norm_weights:
                    nc.vector.tensor_tensor(qs_f32[:], qs_f32[:],
                                            whn_sb[:], op=OP.mult)
                qhb = scr_f.tile([128, D], F32R, tag="sqscr")
                nc.vector.tensor_add(qhb[:], qs_f32[:], peq_bf[:])
                qht = scr_f.tile([128, D], F32R, tag="sqscr")
                for g in range(2):
                    psf = trps.tile([128, 512], F32R, tag="trp")
                    for u in range(4):
                        j = g * 4 + u
                        nc.tensor.transpose(
                            psf[:, u * 128:(u + 1) * 128],
                            qhb[:, j * 128:(j + 1) * 128],
                            idf_sb[:],
                        )
                    nc.vector.tensor_copy(
                        qht[:, g * 512:(g + 1) * 512], psf[:])
                qps = mmps.tile([128, D], F32, tag="mm")
                for j in range(NJ):
                    for h in range(2):
                        nc.tensor.matmul(
                            qps[:, h * 512:(h + 1) * 512],
                            qht[:, j * 128:(j + 1) * 128],
                            wq_l[j][:, h * 512:(h + 1) * 512],
                            start=(j == 0),
                            stop=(j == NJ - 1),
                        )
                qrstd = rms_stats(qps[:])
                qb = scr_bf.tile([128, D], BF16, tag="tmb")
                nc.vector.tensor_scalar(qb[:], qps[:], qrstd[:], None,
                                        OP.mult)
                if apply_norm_weights:
                    nc.vector.tensor_tensor(qb[:], qb[:], wqn_sb[:],
                                            op=OP.mult)
                nc.sync.dma_start(ag_in[:], qb[:])
                nc.gpsimd.collective_compute(
                    "AllGather", OP.bypass,
                    replica_groups=[list(range(NC))],
                    ins=[ag_in.opt()],
                    outs=[ag_out.opt()],
                )

                # ---- history tiles (batch-1 slabs issued while batch-0
                # tiles compute; ring reuse paces them automatically) ----
                for t in range(8):
                    bb, tl = divmod(t, 4)
                    if t < 3:
                        for pr in range(4 * t, 4 * (t + 1)):
                            slab_load(1, pr)
                    fps = mmps.tile([128, D], F32, tag="mm")
                    for h in range(2):
                        for kk in range(NKK):
                            nc.tensor.matmul(
                                fps[:, h * 512:(h + 1) * 512],
                                fc_lhsT(bb, kk, tl),
                                wfc_l[kk][:, h * 512:(h + 1) * 512],
                                start=(kk == 0),
                                stop=(kk == NKK - 1),
                            )
                    rstd = rms_stats(fps[:])
                    fb = scr_bf.tile([128, D], BF16, tag="tmb")
                    nc.vector.tensor_scalar(fb[:], fps[:], rstd[:], None,
                                            OP.mult)
                    if apply_norm_weights:
                        nc.vector.tensor_tensor(fb[:], fb[:], whn_sb[:],
                                                op=OP.mult)
                    for g in range(2):
                        transpose_to(
                            fusedT_bv[bb][:, g * 4:(g + 1) * 4,
                                          tl * 128:(tl + 1) * 128],
                            fb[:],
                            [g * 4 + u for u in range(4)],
                        )

            # -------- phase 2: K^T, V per batch; qT; attention; RS ----------
            with (
                tc.tile_pool(name="ph2w", bufs=1) as ph2w,
            ):
                kT_b = [ph2w.tile([128, NJ * CH], BF16, name=f"kT{b}")
                        for b in range(B)]
                kT_bv = [kT[:].rearrange("p (j t) -> p j t", j=NJ)
                         for kT in kT_b]
                v_b = [ph2w.tile([128, 4 * D], BF16, name=f"v{b}")
                       for b in range(B)]
                qT = ph2w.tile([128, NJ * QT], BF16)
                qT_v = qT[:].rearrange("p (j t) -> p j t", j=NJ)

                def half_cast(dst, srct):
                    nc.scalar.copy(dst[:, 0:512], srct[:, 0:512])
                    nc.vector.tensor_copy(dst[:, 512:1024], srct[:, 512:1024])

                def load_w_slices(src, nm, tag=None):
                    tiles = []
                    for s_ in range(NJ):
                        wst = ph2w.tile([128, D], F32, tag="wst2", bufs=3,
                                        name=f"{nm}st{s_}")
                        eng = nc.sync if s_ % 2 == 0 else nc.scalar
                        eng.dma_start(
                            wst[:], src.ap()[s_ * 128:(s_ + 1) * 128, :])
                        wt = ph2w.tile([128, D], BF16, tag=(tag or nm),
                                       bufs=NJ, name=f"{nm}{s_}")
                        half_cast(wt, wst)
                        tiles.append(wt)
                    return tiles

                wk_l = load_w_slices(wk, "wk")
                pet_bf = ph2w.tile([128, NJ * CH], BF16)     # [d_lo, j*512+tc]
                nc.scalar.dma_start(
                    pet_bf[:].rearrange("p (j t) -> p j t", j=NJ),
                    pet.ap().rearrange("(j p) t -> p j t", p=128))
                pet_v = pet_bf[:].rearrange("p (j t) -> p j t", j=NJ)
                wv_l = load_w_slices(wv, "wv")

                def k_tile(bb, tl):
                    khb = ph2w.tile([128, NJ * 128], BF16, tag="khb", bufs=2)
                    nc.vector.tensor_add(
                        khb[:].rearrange("p (j x) -> p j x", j=NJ),
                        fusedT_bv[bb][:, :, tl * 128:(tl + 1) * 128],
                        pet_v[:, :, tl * 128:(tl + 1) * 128],
                    )
                    kps = mmps.tile([128, D], F32, tag="mm")
                    for h in range(2):
                        for j in range(NJ):
                            nc.tensor.matmul(
                                kps[:, h * 512:(h + 1) * 512],
                                khb[:, j * 128:(j + 1) * 128],
                                wk_l[j][:, h * 512:(h + 1) * 512],
                                start=(j == 0),
                                stop=(j == NJ - 1),
                            )
                    krstd = rms_stats(kps[:])
                    kb = scr_bf.tile([128, D], BF16, tag="tmb")
                    nc.vector.tensor_scalar(kb[:], kps[:], krstd[:], None,
                                            OP.mult)
                    if apply_norm_weights:
                        nc.vector.tensor_tensor(kb[:], kb[:], wkn_sb[:],
                                                op=OP.mult)
                    for g in range(2):
                        transpose_to(
                            kT_bv[bb][:, g * 4:(g + 1) * 4,
                                      tl * 128:(tl + 1) * 128],
                            kb[:],
                            [g * 4 + u for u in range(4)],
                        )

                def v_tile(bb, tl):
                    for h in range(2):
                        vps = scps.tile([128, 512], F32, tag="sc")
                        for j in range(NJ):
                            nc.tensor.matmul(
                                vps[:],
                                fusedT_bv[bb][:, j:j + 1,
                                              tl * 128:(tl + 1) * 128]
                                .rearrange("p j x -> p (j x)"),
                                wv_l[j][:, h * 512:(h + 1) * 512],
                                start=(j == 0),
                                stop=(j == NJ - 1),
                            )
                        nc.vector.tensor_copy(
                            v_b[bb][:, tl * D + h * 512: tl * D + h * 512 + 512],
                            vps[:])

                def attn_tile(i, rs_buf, row0, after=None):
                    bchunk = 0 if i < 4 else 1
                    sps = scps.tile([128, 512], F32, tag="sc")
                    first_mm = None
                    for j in range(NJ):
                        mm_i = nc.tensor.matmul(
                            sps[:],
                            qT[:, j * QT + i * 128: j * QT + (i + 1) * 128],
                            kT_b[bchunk][:, j * CH:(j + 1) * CH],
                            start=(j == 0),
                            stop=(j == NJ - 1),
                        )
                        if first_mm is None:
                            first_mm = mm_i
                            if after is not None:
                                tile.add_dep_helper(
                                    first_mm.ins,
                                    after.ins,
                                    sync=False,
                                    reason="order attn after prior RS inputs")
                    mb = ph2w.tile([128, CH], F16, tag="mb", bufs=2)
                    nc.vector.tensor_scalar(mb[:], iota_sb[:],
                                            thr_sb[:, i:i + 1], MASK_NEG,
                                            OP.is_gt, OP.mult)
                    sm = ph2w.tile([128, CH], F32, tag="sm", bufs=2)
                    nc.vector.tensor_add(sm[:], sps[:], mb[:])
                    o_sb = ph2w.tile([128, D + 1], F16, tag="osb", bufs=2)
                    lacc = stat.tile([128, 1], F32, tag="lacc")
                    probs = ph2w.tile([128, CH], BF16, tag="probs", bufs=2)
                    nc.scalar.activation(probs[:], sm[:], AF.Exp, scale=SCALE,
                                         bias=ebias_sb[:], accum_out=lacc[:])
                    nc.vector.tensor_copy(o_sb[:, D:D + 1], lacc[:])
                    pps = trps.tile([128, 512], BF16, tag="trp")
                    for u in range(4):
                        nc.tensor.transpose(
                            pps[:, u * 128:(u + 1) * 128],
                            probs[:, u * 128:(u + 1) * 128],
                            id_sb[:],
                        )
                    pT = ph2w.tile([128, 512], BF16, tag="pT", bufs=2)
                    nc.vector.tensor_copy(pT[:], pps[:])
                    ops_ = mmps.tile([128, D], F32, tag="mm")
                    for h in range(2):
                        for u in range(4):
                            nc.tensor.matmul(
                                ops_[:, h * 512:(h + 1) * 512],
                                pT[:, u * 128:(u + 1) * 128],
                                v_b[bchunk][:, u * D + h * 512:
                                            u * D + h * 512 + 512],
                                start=(u == 0),
                                stop=(u == 3),
                            )
                    nc.vector.tensor_copy(o_sb[:, 0:D], ops_[:])
                    return nc.sync.dma_start(rs_buf[row0:row0 + 128, :],
                                             o_sb[:])

                # qT gather from AG output.
                # batch0 tiles (i<4): queries 128i+k owned 64-apiece by cores
                # 2i, 2i+1 (first 64 rows of their AG block).
                # batch1 tiles: 32-query blocks; tiles 4,6 from cores 0-3,
                # tiles 5,7 from cores 4-7; rows 64:96 (first half) or
                # 96:128 (second half) of the AG block.
                for i in range(NC):
                    qg = ph2w.tile([128, D], BF16, tag="qg", bufs=2)
                    if i < 4:
                        r0 = (2 * i) * 128
                        r1 = (2 * i + 1) * 128
                        nc.sync.dma_start(qg[0:64, :], ag_out[r0:r0 + 64, :])
                        nc.sync.dma_start(qg[64:128, :], ag_out[r1:r1 + 64, :])
                    else:
                        half = (i - 4) // 2          # 0 for tiles 4,5; 1 for 6,7
                        cbase = 4 * ((i - 4) % 2)    # cores 0-3 or 4-7
                        srow = 64 + 32 * half
                        for m in range(4):
                            r = (cbase + m) * 128 + srow
                            nc.sync.dma_start(
                                qg[32 * m:32 * m + 32, :],
                                ag_out[r:r + 32, :])
                    for g in range(2):
                        transpose_to(
                            qT_v[:, g * 4:(g + 1) * 4, i * 128:(i + 1) * 128],
                            qg[:],
                            [g * 4 + u for u in range(4)],
                        )

                for tl in range(4):
                    k_tile(0, tl)
                    v_tile(0, tl)
                wo_l = load_w_slices(wo, "wo", tag="wk")

                last_dma = None
                for i in range(4):
                    last_dma = attn_tile(i, rs_inA, i * 128)
                nc.gpsimd.collective_compute(
                    "ReduceScatter", OP.add,
                    replica_groups=[list(range(NC))],
                    ins=[rs_inA.opt()],
                    outs=[rs_outA.opt()],
                )
                for tl in range(4):
                    k_tile(1, tl)
                    v_tile(1, tl)
                for i in (4, 5):
                    last_dma = attn_tile(i, rs_inB1, (i - 4) * 128,
                                         after=last_dma)
                nc.gpsimd.collective_compute(
                    "ReduceScatter", OP.add,
                    replica_groups=[list(range(NC))],
                    ins=[rs_inB1.opt()],
                    outs=[rs_outB1.opt()],
                )
                for i in (6, 7):
                    last_dma = attn_tile(i, rs_inB2, (i - 6) * 128,
                                         after=last_dma)
                nc.gpsimd.collective_compute(
                    "ReduceScatter", OP.add,
                    replica_groups=[list(range(NC))],
                    ins=[rs_inB2.opt()],
                    outs=[rs_outB2.opt()],
                )

                # ---------------- epilogue for own 128 queries --------------
                fo = ph2w.tile([QPC, D + 1], F16, tag="fo", bufs=1)
                nc.sync.dma_start(fo[0:64, :], rs_outA[:])
                nc.sync.dma_start(fo[64:96, :], rs_outB1[:])
                nc.sync.dma_start(fo[96:128, :], rs_outB2[:])
                linv = stat.tile([128, 1], F32, tag="linv")
                nc.vector.reciprocal(linv[:], fo[:, D:D + 1])
                ao = scr_bf.tile([128, D], BF16, tag="tmb")
                nc.vector.tensor_scalar(ao[:], fo[:, 0:D], linv[:], None,
                                        OP.mult)
                aoT = scr_bf.tile([128, D], BF16, tag="tmb")
                aoT_v = aoT[:].rearrange("p (g x) -> p g x", g=2)
                for g in range(2):
                    transpose_to(
                        aoT_v[:, g:g + 1, :].rearrange("p g x -> p (g x)")
                        .rearrange("p (u x) -> p u x", u=4),
                        ao[:],
                        [g * 4 + u for u in range(4)],
                    )
                zps = mmps.tile([128, D], F32, tag="mm")
                for h in range(2):
                    for j in range(NJ):
                        nc.tensor.matmul(
                            zps[:, h * 512:(h + 1) * 512],
                            aoT[:, j * 128:(j + 1) * 128],
                            wo_l[j][:, h * 512:(h + 1) * 512],
                            start=(j == 0),
                            stop=(j == NJ - 1),
                        )
                hh = scr_f.tile([128, D], F32, tag="sqscr")
                nc.vector.tensor_add(hh[:], qs_f32[:], zps[:])
                orstd = rms_stats(hh[:])
                yv = scr_f.tile([128, D], F32, tag="sqscr")
                nc.vector.tensor_scalar(yv[:], hh[:], orstd[:], None, OP.mult)
                if apply_norm_weights:
                    nc.vector.tensor_tensor(yv[:], yv[:], won_sb[:],
                                            op=OP.mult)
                nc.sync.dma_start(out[:], yv[:])

    nc.compile()
    return nc


def _pe_table():
    half = D // 2
    inv_freq = np.exp(np.arange(half, dtype=np.float32)
                      * (-math.log(10000.0) / half))
    ang = np.arange(S, dtype=np.float32)[:, None] * inv_freq
    return np.concatenate([np.sin(ang), np.cos(ang)], axis=-1).astype(np.float32)


def _core_gidx(c):
    """Global query indices owned by core c: 64 batch-0 queries
    [64c, 64c+64), then batch-1 queries [32c, 32c+32) and
    [256+32c, 256+32c+32) — matching the RS_A / RS_B1 / RS_B2 splits."""
    return np.concatenate([
        np.arange(64 * c, 64 * c + 64),
        N + np.arange(32 * c, 32 * c + 32),
        N + 256 + np.arange(32 * c, 32 * c + 32),
    ])


def make_in_maps(np_inputs, apply_w=False):
    hid = np.asarray(np_inputs["hidden_states"], np.float32)
    pos = np.asarray(np_inputs["context_positions"])
    Wfc = np.ascontiguousarray(np.asarray(np_inputs["W_fc"], np.float32))
    Wq = np.ascontiguousarray(np.asarray(np_inputs["Wq"], np.float32))
    Wk = np.ascontiguousarray(np.asarray(np_inputs["Wk"], np.float32))
    Wv = np.ascontiguousarray(np.asarray(np_inputs["Wv"], np.float32))
    Wo = np.ascontiguousarray(np.asarray(np_inputs["Wo"], np.float32))

    x = hid.reshape(B, S, LD)
    p = np.clip(pos.astype(np.int64), 0, S - 1)
    p_flat = p.reshape(QT)
    PE = _pe_table()

    iota_np = np.tile(np.arange(CH, dtype=np.float32), (128, 1))
    ident_np = np.eye(128, dtype=np.float32).astype(ml_dtypes.bfloat16)
    identf_np = np.eye(128, dtype=np.float32)

    in_maps = []
    for c in range(NC):
        sl = slice(c * CH, (c + 1) * CH)
        xT_c = np.ascontiguousarray(
            np.concatenate([x[0, sl], x[1, sl]], axis=0).T)
        gidx = _core_gidx(c)
        xqT_a = np.ascontiguousarray(x[gidx // N, p_flat[gidx]].T)
        peq_a = np.ascontiguousarray(PE[p_flat[gidx]]).astype(ml_dtypes.bfloat16)
        pet_a = np.ascontiguousarray(PE[sl].T).astype(ml_dtypes.bfloat16)
        thr_a = np.ascontiguousarray(
            (p_flat.astype(np.float32) - c * CH).reshape(NC, 128).T)
        m = {
            "xT": xT_c, "xqT": xqT_a,
            "wfc": Wfc, "wq": Wq, "wk": Wk, "wv": Wv, "wo": Wo,
            "pet": pet_a, "peq": peq_a, "thr": thr_a,
            "iota": iota_np, "ident": ident_np, "identf": identf_np,
        }
        if apply_w:
            m["whn"] = np.tile(np.asarray(np_inputs["w_hidden_norm"], np.float32), (128, 1))
            m["wqn"] = np.tile(np.asarray(np_inputs["w_q_norm"], np.float32), (128, 1))
            m["wkn"] = np.tile(np.asarray(np_inputs["w_k_norm"], np.float32), (128, 1))
            m["won"] = np.tile(np.asarray(np_inputs["w_out_norm"], np.float32), (128, 1))
        in_maps.append(m)
    return in_maps


def assemble_out(results):
    y = np.zeros((QT, D), np.float32)
    for c in range(NC):
        y[_core_gidx(c)] = results[c]["out"]
    return y.reshape(B, N, D)


def kernel(**inputs) -> np.ndarray:
    w_h = np.asarray(inputs["w_hidden_norm"], np.float32)
    w_q = np.asarray(inputs["w_q_norm"], np.float32)
    w_k = np.asarray(inputs["w_k_norm"], np.float32)
    w_o = np.asarray(inputs["w_out_norm"], np.float32)
    apply_w = not (np.all(w_h == 1) and np.all(w_q == 1)
                   and np.all(w_k == 1) and np.all(w_o == 1))

    key = ("nc", apply_w)
    if key not in _CACHE:
        _CACHE[key] = _build(apply_w)
    nc = _CACHE[key]

    in_maps = make_in_maps(inputs, apply_w)

    trace = os.environ.get("KERNEL_TRACE", "0") == "1"
    if trace:
        try:
            import axon_prof
            axon_prof.install()
        except Exception:
            trace = False
    res = run_bass_kernel_spmd(nc, in_maps, list(range(NC)), trace=trace)
    global LAST_EXEC_NS
    LAST_EXEC_NS = res.exec_time_ns

    return assemble_out(res.results).astype(np.float32)


LAST_EXEC_NS = None


# revision 38
# speedup vs baseline: 1.0605x; 1.0112x over previous
"""Trainium2 Bass kernel for nn_CHSHistoryCrossAttentionFusion (8 NeuronCores, SPMD).

Decomposition (hardcoded for B=2, S=4096, L=3, D=1024, N=512, 8 cores):
  - History sequence-sharded: core c owns key positions [c*512, (c+1)*512) of
    each batch; it computes its chunk of fused/K/V from its x chunk.
  - Queries sharded 8-way for the Q path (see _core_gidx); an AllGather
    replicates Q (bf16, small) so every core scores all 1024 queries against
    its own K/V chunk.
  - Flash-style partial softmax per chunk WITHOUT max subtraction (Q/K are
    RMS-normalized so scores are bounded); causal mask applied additively
    before exp; exp carries a constant -ln(256) prescale so the (o,l)
    partials fit fp16.  Partials combine via fp16 ReduceScatter-adds:
    one RS for batch 0 (fully overlapped with batch-1 work) and TWO
    half-size RS for batch 1 so only the last 256-query chunk's wire time
    is exposed at the tail.
  - x is supplied pre-transposed by the host (layout-only change) so the
    fc contraction consumes it directly as the stationary operand — no
    PE transposes on the input side of phase 1.
  - Phase-1 matmuls (fc + Q-projection) run with BOTH operands typed
    float32r: the PE streams f32r at full (1 elem/cycle) moving rate for
    N>=512, so x/W_fc/Wq need NO on-chip f32->bf16 casts at all — the
    entire stage+cast+ring latency chain of the front end is gone and the
    AllGather fires ~60us earlier.  Phase-2 matmuls stay bf16 (fp32 acc)
    with weights staged+half-cast (scalar+vector in parallel) under
    relaxed deadlines.
  - DMA queues: weights and x slabs alternate across the scalar and sync
    HWDGE queues; x rides as [128, 2x512] kk-pair slabs with 2KB
    contiguous rows (fast descriptors).
Host-side work is layout/indexing only.
"""

import math
import os

import numpy as np

try:
    import ml_dtypes
except ImportError:  # pragma: no cover
    ml_dtypes = None

import concourse.bacc as bacc
import concourse.mybir as mybir
import concourse.tile as tile
import concourse.tile_utils as tile_utils
from concourse.bass_utils import run_bass_kernel_spmd

# cayman has 208 KiB/partition usable; the default constant leaves 16 KiB idle
tile_utils.max_sbuf_usage = 208 * 1024

F32 = mybir.dt.float32
F16 = mybir.dt.float16
BF16 = mybir.dt.bfloat16
AF = mybir.ActivationFunctionType
OP = mybir.AluOpType

B, S, L, D = 2, 4096, 3, 1024
N = 512
NC = 8
CH = S // NC              # 512 keys per batch per core
LD = L * D                # 3072
QT = B * N                # 1024 global queries
QPC = QT // NC            # 128 queries per core (64 b0 + 2x32 b1)
NKK = LD // 128           # 24 contraction slices over 3072
NJ = D // 128             # 8 contraction slices over 1024
RMS_EPS = 1e-6
SCALE = D ** -0.5
MASK_NEG = -60000.0
EXP_BIAS = -math.log(256.0)

_CACHE = {}


def _build(apply_norm_weights: bool):
    nc = bacc.Bacc("TRN2", target_bir_lowering=False, num_devices=NC)

    # ---------------- I/O ----------------
    F32R = mybir.dt.float32r
    # x transposed on host: [LD, B*CH] (cols 0:512 batch0, 512:1024 batch1).
    # Typed float32r end-to-end (raw f32 bits): PE streams f32r at full rate
    # as the moving operand (N>=256) and the verifier requires the producer
    # chain to be f32r-typed.
    xT = nc.dram_tensor("xT", [LD, B * CH], F32R, kind="ExternalInput")
    xqT = nc.dram_tensor("xqT", [LD, QPC], F32R, kind="ExternalInput")
    wfc = nc.dram_tensor("wfc", [LD, D], F32R, kind="ExternalInput")
    wq = nc.dram_tensor("wq", [D, D], F32R, kind="ExternalInput")
    wk = nc.dram_tensor("wk", [D, D], F32, kind="ExternalInput")
    wv = nc.dram_tensor("wv", [D, D], F32, kind="ExternalInput")
    wo = nc.dram_tensor("wo", [D, D], F32, kind="ExternalInput")
    identf = nc.dram_tensor("identf", [128, 128], F32R, kind="ExternalInput")
    # positional tables are host-synthesized constants -> ship as bf16
    pet = nc.dram_tensor("pet", [D, CH], BF16, kind="ExternalInput")
    peq = nc.dram_tensor("peq", [QPC, D], BF16, kind="ExternalInput")
    thr = nc.dram_tensor("thr", [128, NC], F32, kind="ExternalInput")
    iota = nc.dram_tensor("iota", [128, CH], F32, kind="ExternalInput")
    ident = nc.dram_tensor("ident", [128, 128], BF16, kind="ExternalInput")
    if apply_norm_weights:
        whn = nc.dram_tensor("whn", [128, D], F32, kind="ExternalInput")
        wqn = nc.dram_tensor("wqn", [128, D], F32, kind="ExternalInput")
        wkn = nc.dram_tensor("wkn", [128, D], F32, kind="ExternalInput")
        won = nc.dram_tensor("won", [128, D], F32, kind="ExternalInput")
    out = nc.dram_tensor("out", [QPC, D], F32, kind="ExternalOutput")

    # DRAM-side transposed views for the 3D gather loads
    xT_v = xT.ap().rearrange("(k p) n -> p k n", p=128)      # [128, 24, 1024]
    xqT_v = xqT.ap().rearrange("(k p) n -> p k n", p=128)    # [128, 24, 128]

    with tile.TileContext(nc) as tc:
        with (
            tc.tile_pool(name="dram", bufs=1, space="DRAM") as dram,
            tc.tile_pool(name="const", bufs=1) as constp,
            tc.tile_pool(name="stat", bufs=6) as stat,
            tc.tile_pool(name="base", bufs=1) as base,
            tc.tile_pool(name="scr_bf", bufs=3) as scr_bf,
            tc.tile_pool(name="scr_f", bufs=2) as scr_f,
            tc.tile_pool(name="mmps", bufs=2, space="PSUM") as mmps,
            tc.tile_pool(name="trps", bufs=2, space="PSUM") as trps,
            tc.tile_pool(name="scps", bufs=2, space="PSUM") as scps,
        ):
            # collective bounce buffers
            ag_in = dram.tile([QPC, D], BF16)
            ag_out = dram.tile([QT, D], BF16, addr_space="Shared")
            rs_inA = dram.tile([N, D + 1], F16)
            rs_outA = dram.tile([N // NC, D + 1], F16)
            rs_inB1 = dram.tile([N // 2, D + 1], F16)
            rs_outB1 = dram.tile([N // (2 * NC), D + 1], F16)
            rs_inB2 = dram.tile([N // 2, D + 1], F16)
            rs_outB2 = dram.tile([N // (2 * NC), D + 1], F16)

            # constants (bulk queue)
            id_sb = constp.tile([128, 128], BF16)
            nc.scalar.dma_start(id_sb[:], ident[:])
            idf_sb = constp.tile([128, 128], F32R)
            nc.scalar.dma_start(idf_sb[:], identf[:])
            iota_sb = constp.tile([128, CH], F32)
            nc.scalar.dma_start(iota_sb[:], iota[:])
            thr_sb = constp.tile([128, NC], F32)
            nc.scalar.dma_start(thr_sb[:], thr[:])
            eps_sb = constp.tile([128, 1], F32)
            nc.vector.memset(eps_sb[:], RMS_EPS)
            ebias_sb = constp.tile([128, 1], F32)
            nc.vector.memset(ebias_sb[:], EXP_BIAS)
            if apply_norm_weights:
                whn_sb = constp.tile([128, D], F32)
                nc.scalar.dma_start(whn_sb[:], whn[:])
                wqn_sb = constp.tile([128, D], F32)
                nc.scalar.dma_start(wqn_sb[:], wqn[:])
                wkn_sb = constp.tile([128, D], F32)
                nc.scalar.dma_start(wkn_sb[:], wkn[:])
                won_sb = constp.tile([128, D], F32)
                nc.scalar.dma_start(won_sb[:], won[:])

            # persistent activations (per-batch splits for fine-grained deps)
            fusedT_b = [base.tile([128, NJ * (4 * 128)], BF16, name=f"fusedT{b}")
                        for b in range(B)]
            fusedT_bv = [fT[:].rearrange("p (j t) -> p j t", j=NJ)
                         for fT in fusedT_b]
            qs_f32 = base.tile([QPC, D], F32)

            def rms_stats(src_ap):
                sq = scr_f.tile([128, D], F32, tag="sqscr")
                ssq = stat.tile([128, 1], F32, tag="ssq")
                nc.scalar.activation(sq[:], src_ap, AF.Square, accum_out=ssq[:])
                std = stat.tile([128, 1], F32, tag="std")
                nc.scalar.activation(std[:], ssq[:], AF.Sqrt, scale=1.0 / D,
                                     bias=eps_sb[:])
                rstd = stat.tile([128, 1], F32, tag="rstd")
                nc.vector.reciprocal(rstd[:], std[:])
                return rstd

            def transpose_to(dst_ap_3d, src_tile_ap, jlist):
                """PE-transpose 128x128 blocks into dst 3d view [128,len,128]."""
                ps = trps.tile([128, 512], BF16, tag="trp")
                for u, j in enumerate(jlist):
                    nc.tensor.transpose(
                        ps[:, u * 128:(u + 1) * 128],
                        src_tile_ap[:, j * 128:(j + 1) * 128],
                        id_sb[:],
                    )
                nc.vector.tensor_copy(
                    dst_ap_3d,
                    ps[:].rearrange("p (u x) -> p u x", u=len(jlist)),
                )

            # ---------------- phase 1: fc matmul for 9 token tiles ----------
            with (
                tc.tile_pool(name="ph1w", bufs=1) as ph1w,
                tc.tile_pool(name="ph1x", bufs=2) as ph1x,
            ):
                # Queue plan (FIFO per HWDGE queue, ~210 GB/s each).  x tiles
                # are interleaved with the weight loads on BOTH queues in
                # consumption order so the PE starts fc matmuls ~15us in and
                # never starves:
                #   sync  : xqT, x-q0, wfc[12:24], x-q2, wk[0:4], x-q4, x-q6,
                #           qT gather, RS payloads
                #   scalar: peq, wfc[0:12], x-q1, wq, x-q3, wk[4:8], pet,
                #           x-q5, x-q7, wv, wo
                # Cast engines: scalar takes wfc-A + odd x tiles; vector takes
                # wfc-B, wq, wk + even x tiles.  pet/peq ship as bf16.

                # sync queue head: the query x slab (512B descriptors --
                # slow-ish, but it's first and short); consumed directly as
                # the f32r stationary operand, no cast
                xqf = ph1x.tile([128, NKK * 128], F32R, tag="xqf", bufs=1)
                nc.sync.dma_start(
                    xqf[:].rearrange("p (k n) -> p k n", k=NKK), xqT_v)
                xq_r = xqf[:]
                # scalar queue head: peq (bf16, direct)
                peq_bf = ph1w.tile([QPC, D], BF16)
                nc.scalar.dma_start(peq_bf[:], peq[:])

                # W_fc: direct-resident float32r tiles, no staging/casts --
                # ready at pure queue rate, alternating queues per slice
                wfc_l = []
                for s_ in range(NKK):
                    wt = ph1w.tile([128, D], F32R, tag="wfc", bufs=NKK,
                                   name=f"wfc{s_}")
                    eng = nc.scalar if s_ % 2 else nc.sync
                    eng.dma_start(wt[:], wfc.ap()[s_ * 128:(s_ + 1) * 128, :])
                    wfc_l.append(wt)

                # Wq: 4-deep float32r ring (transient; consumed once by qps)
                wq_l = []
                for s_ in range(NJ):
                    wt = ph1w.tile([128, D], F32R, tag="wq", bufs=3,
                                   name=f"wq{s_}")
                    nc.scalar.dma_start(
                        wt[:], wq.ap()[s_ * 128:(s_ + 1) * 128, :])
                    wq_l.append(wt)

                # history x as kk-pair SLABS spanning all tokens of both
                # batches' chunks: [128, 2, 512] f32 slices of xT with 2KB
                # contiguous rows (fast descriptors).  12 slab-pairs per
                # batch resident as bf16; batch-1 slabs ring-reuse batch-0
                # slots once fc b0 has consumed them.
                NP = NKK // 2            # 12 kk-pairs
                xsl_b = [[ph1x.tile([128, 2 * CH], F32R, tag="xsl",
                                    bufs=NP, name=f"xsl{bb}_{pr}")
                          for pr in range(NP)] for bb in range(2)]

                def slab_load(bb, pr):
                    eng = nc.sync if pr % 2 == 0 else nc.scalar
                    eng.dma_start(
                        xsl_b[bb][pr][:].rearrange("p (k n) -> p k n", k=2),
                        xT_v[:, 2 * pr:2 * pr + 2, bb * CH:(bb + 1) * CH])

                def fc_lhsT(bb, pr_kk, tl):
                    pr, sub = divmod(pr_kk, 2)
                    off = sub * CH + tl * 128
                    return xsl_b[bb][pr][:, off:off + 128]

                for pr in range(NP):
                    slab_load(0, pr)

                # ---- query tile first (early AG) ----
                fps = mmps.tile([128, D], F32, tag="mm")
                for h in range(2):
                    for kk in range(NKK):
                        nc.tensor.matmul(
                            fps[:, h * 512:(h + 1) * 512],
                            xq_r[:, kk * 128:(kk + 1) * 128],
                            wfc_l[kk][:, h * 512:(h + 1) * 512],
                            start=(kk == 0),
                            stop=(kk == NKK - 1),
                        )
                rstd = rms_stats(fps[:])
                fb = scr_bf.tile([128, D], BF16, tag="tmb")
                nc.vector.tensor_scalar(fb[:], fps[:], rstd[:], None, OP.mult)
                if apply_norm_weights:
                    nc.vector.tensor_tensor(fb[:], fb[:], whn_sb[:],
                                            op=OP.mult)
                nc.vector.tensor_scalar(qs_f32[:], fps[:], rstd[:],
                                        None, OP.mult)
                if apply_norm_weights:
                    nc.vector.tensor_tensor(qs_f32[:], qs_f32[:],
                                            whn_sb[:], op=OP.mult)
                qhb = scr_f.tile([128, D], F32R, tag="sqscr")
                nc.vector.tensor_add(qhb[:], qs_f32[:], peq_bf[:])
                qht = scr_f.tile([128, D], F32R, tag="sqscr")
                for g in range(2):
                    psf = trps.tile([128, 512], F32R, tag="trp")
                    for u in range(4):
                        j = g * 4 + u
                        nc.tensor.transpose(
                            psf[:, u * 128:(u + 1) * 128],
                            qhb[:, j * 128:(j + 1) * 128],
                            idf_sb[:],
                        )
                    nc.vector.tensor_copy(
                        qht[:, g * 512:(g + 1) * 512], psf[:])
                qps = mmps.tile([128, D], F32, tag="mm")
                for j in range(NJ):
                    for h in range(2):
                        nc.tensor.matmul(
                            qps[:, h * 512:(h + 1) * 512],
                            qht[:, j * 128:(j + 1) * 128],
                            wq_l[j][:, h * 512:(h + 1) * 512],
                            start=(j == 0),
                            stop=(j == NJ - 1),
                        )
                qrstd = rms_stats(qps[:])
                qb = scr_bf.tile([128, D], BF16, tag="tmb")
                nc.vector.tensor_scalar(qb[:], qps[:], qrstd[:], None,
                                        OP.mult)
                if apply_norm_weights:
                    nc.vector.tensor_tensor(qb[:], qb[:], wqn_sb[:],
                                            op=OP.mult)
                nc.sync.dma_start(ag_in[:], qb[:])
                nc.gpsimd.collective_compute(
                    "AllGather", OP.bypass,
                    replica_groups=[list(range(NC))],
                    ins=[ag_in.opt()],
                    outs=[ag_out.opt()],
                )

                # ---- history tiles (batch-1 slabs issued while batch-0
                # tiles compute; ring reuse paces them automatically) ----
                for t in range(8):
                    bb, tl = divmod(t, 4)
                    if t < 3:
                        for pr in range(4 * t, 4 * (t + 1)):
                            slab_load(1, pr)
                    fps = mmps.tile([128, D], F32, tag="mm")
                    for h in range(2):
                        for kk in range(NKK):
                            nc.tensor.matmul(
                                fps[:, h * 512:(h + 1) * 512],
                                fc_lhsT(bb, kk, tl),
                                wfc_l[kk][:, h * 512:(h + 1) * 512],
                                start=(kk == 0),
                                stop=(kk == NKK - 1),
                            )
                    rstd = rms_stats(fps[:])
                    fb = scr_bf.tile([128, D], BF16, tag="tmb")
                    nc.vector.tensor_scalar(fb[:], fps[:], rstd[:], None,
                                            OP.mult)
                    if apply_norm_weights:
                        nc.vector.tensor_tensor(fb[:], fb[:], whn_sb[:],
                                                op=OP.mult)
                    for g in range(2):
                        transpose_to(
                            fusedT_bv[bb][:, g * 4:(g + 1) * 4,
                                          tl * 128:(tl + 1) * 128],
                            fb[:],
                            [g * 4 + u for u in range(4)],
                        )

            # -------- phase 2: K^T, V per batch; qT; attention; RS ----------
            with (
                tc.tile_pool(name="ph2w", bufs=1) as ph2w,
            ):
                kT_b = [ph2w.tile([128, NJ * CH], BF16, name=f"kT{b}")
                        for b in range(B)]
                kT_bv = [kT[:].rearrange("p (j t) -> p j t", j=NJ)
                         for kT in kT_b]
                v_b = [ph2w.tile([128, 4 * D], BF16, name=f"v{b}")
                       for b in range(B)]
                qT = ph2w.tile([128, NJ * QT], BF16)
                qT_v = qT[:].rearrange("p (j t) -> p j t", j=NJ)

                def half_cast(dst, srct):
                    nc.scalar.copy(dst[:, 0:512], srct[:, 0:512])
                    nc.vector.tensor_copy(dst[:, 512:1024], srct[:, 512:1024])

                def load_w_slices(src, nm, tag=None):
                    tiles = []
                    for s_ in range(NJ):
                        wst = ph2w.tile([128, D], F32, tag="wst2", bufs=3,
                                        name=f"{nm}st{s_}")
                        eng = nc.sync if s_ % 2 == 0 else nc.scalar
                        eng.dma_start(
                            wst[:], src.ap()[s_ * 128:(s_ + 1) * 128, :])
                        wt = ph2w.tile([128, D], BF16, tag=(tag or nm),
                                       bufs=NJ, name=f"{nm}{s_}")
                        half_cast(wt, wst)
                        tiles.append(wt)
                    return tiles

                wk_l = load_w_slices(wk, "wk")
                pet_bf = ph2w.tile([128, NJ * CH], BF16)     # [d_lo, j*512+tc]
                nc.scalar.dma_start(
                    pet_bf[:].rearrange("p (j t) -> p j t", j=NJ),
                    pet.ap().rearrange("(j p) t -> p j t", p=128))
                pet_v = pet_bf[:].rearrange("p (j t) -> p j t", j=NJ)
                wv_l = load_w_slices(wv, "wv")

                def k_tile(bb, tl):
                    khb = ph2w.tile([128, NJ * 128], BF16, tag="khb", bufs=2)
                    nc.vector.tensor_add(
                        khb[:].rearrange("p (j x) -> p j x", j=NJ),
                        fusedT_bv[bb][:, :, tl * 128:(tl + 1) * 128],
                        pet_v[:, :, tl * 128:(tl + 1) * 128],
                    )
                    kps = mmps.tile([128, D], F32, tag="mm")
                    for h in range(2):
                        for j in range(NJ):
                            nc.tensor.matmul(
                                kps[:, h * 512:(h + 1) * 512],
                                khb[:, j * 128:(j + 1) * 128],
                                wk_l[j][:, h * 512:(h + 1) * 512],
                                start=(j == 0),
                                stop=(j == NJ - 1),
                            )
                    krstd = rms_stats(kps[:])
                    kb = scr_bf.tile([128, D], BF16, tag="tmb")
                    nc.vector.tensor_scalar(kb[:], kps[:], krstd[:], None,
                                            OP.mult)
                    if apply_norm_weights:
                        nc.vector.tensor_tensor(kb[:], kb[:], wkn_sb[:],
                                                op=OP.mult)
                    for g in range(2):
                        transpose_to(
                            kT_bv[bb][:, g * 4:(g + 1) * 4,
                                      tl * 128:(tl + 1) * 128],
                            kb[:],
                            [g * 4 + u for u in range(4)],
                        )

                def v_tile(bb, tl):
                    for h in range(2):
                        vps = scps.tile([128, 512], F32, tag="sc")
                        for j in range(NJ):
                            nc.tensor.matmul(
                                vps[:],
                                fusedT_bv[bb][:, j:j + 1,
                                              tl * 128:(tl + 1) * 128]
                                .rearrange("p j x -> p (j x)"),
                                wv_l[j][:, h * 512:(h + 1) * 512],
                                start=(j == 0),
                                stop=(j == NJ - 1),
                            )
                        nc.vector.tensor_copy(
                            v_b[bb][:, tl * D + h * 512: tl * D + h * 512 + 512],
                            vps[:])

                def attn_tile(i, rs_buf, row0, after=None):
                    bchunk = 0 if i < 4 else 1
                    sps = scps.tile([128, 512], F32, tag="sc")
                    first_mm = None
                    for j in range(NJ):
                        mm_i = nc.tensor.matmul(
                            sps[:],
                            qT[:, j * QT + i * 128: j * QT + (i + 1) * 128],
                            kT_b[bchunk][:, j * CH:(j + 1) * CH],
                            start=(j == 0),
                            stop=(j == NJ - 1),
                        )
                        if first_mm is None:
                            first_mm = mm_i
                            if after is not None:
                                tile.add_dep_helper(
                                    first_mm.ins,
                                    after.ins,
                                    sync=False,
                                    reason="order attn after prior RS inputs")
                    mb = ph2w.tile([128, CH], F16, tag="mb", bufs=2)
                    nc.vector.tensor_scalar(mb[:], iota_sb[:],
                                            thr_sb[:, i:i + 1], MASK_NEG,
                                            OP.is_gt, OP.mult)
                    sm = ph2w.tile([128, CH], F32, tag="sm", bufs=2)
                    nc.vector.tensor_add(sm[:], sps[:], mb[:])
                    o_sb = ph2w.tile([128, D + 1], F16, tag="osb", bufs=2)
                    lacc = stat.tile([128, 1], F32, tag="lacc")
                    probs = ph2w.tile([128, CH], BF16, tag="probs", bufs=2)
                    nc.scalar.activation(probs[:], sm[:], AF.Exp, scale=SCALE,
                                         bias=ebias_sb[:], accum_out=lacc[:])
                    nc.vector.tensor_copy(o_sb[:, D:D + 1], lacc[:])
                    pps = trps.tile([128, 512], BF16, tag="trp")
                    for u in range(4):
                        nc.tensor.transpose(
                            pps[:, u * 128:(u + 1) * 128],
                            probs[:, u * 128:(u + 1) * 128],
                            id_sb[:],
                        )
                    pT = ph2w.tile([128, 512], BF16, tag="pT", bufs=2)
                    nc.vector.tensor_copy(pT[:], pps[:])
                    ops_ = mmps.tile([128, D], F32, tag="mm")
                    for h in range(2):
                        for u in range(4):
                            nc.tensor.matmul(
                                ops_[:, h * 512:(h + 1) * 512],
                                pT[:, u * 128:(u + 1) * 128],
                                v_b[bchunk][:, u * D + h * 512:
                                            u * D + h * 512 + 512],
                                start=(u == 0),
                                stop=(u == 3),
                            )
                    nc.vector.tensor_copy(o_sb[:, 0:D], ops_[:])
                    return nc.sync.dma_start(rs_buf[row0:row0 + 128, :],
                                             o_sb[:])

                # qT gather from AG output.
                # batch0 tiles (i<4): queries 128i+k owned 64-apiece by cores
                # 2i, 2i+1 (first 64 rows of their AG block).
                # batch1 tiles: 32-query blocks; tiles 4,6 from cores 0-3,
                # tiles 5,7 from cores 4-7; rows 64:96 (first half) or
                # 96:128 (second half) of the AG block.
                for i in range(NC):
                    qg = ph2w.tile([128, D], BF16, tag="qg", bufs=2)
                    if i < 4:
                        r0 = (2 * i) * 128
                        r1 = (2 * i + 1) * 128
                        nc.sync.dma_start(qg[0:64, :], ag_out[r0:r0 + 64, :])
                        nc.sync.dma_start(qg[64:128, :], ag_out[r1:r1 + 64, :])
                    else:
                        half = (i - 4) // 2          # 0 for tiles 4,5; 1 for 6,7
                        cbase = 4 * ((i - 4) % 2)    # cores 0-3 or 4-7
                        srow = 64 + 32 * half
                        for m in range(4):
                            r = (cbase + m) * 128 + srow
                            nc.sync.dma_start(
                                qg[32 * m:32 * m + 32, :],
                                ag_out[r:r + 32, :])
                    for g in range(2):
                        transpose_to(
                            qT_v[:, g * 4:(g + 1) * 4, i * 128:(i + 1) * 128],
                            qg[:],
                            [g * 4 + u for u in range(4)],
                        )

                for tl in range(4):
                    k_tile(0, tl)
                    v_tile(0, tl)
                wo_l = load_w_slices(wo, "wo", tag="wk")

                last_dma = None
                for i in range(4):
                    last_dma = attn_tile(i, rs_inA, i * 128)
                nc.gpsimd.collective_compute(
                    "ReduceScatter", OP.add,
                    replica_groups=[list(range(NC))],
                    ins=[rs_inA.opt()],
                    outs=[rs_outA.opt()],
                )
                for tl in range(4):
                    k_tile(1, tl)
                    v_tile(1, tl)
                for i in (4, 5):
                    last_dma = attn_tile(i, rs_inB1, (i - 4) * 128,
                                         after=last_dma)
                nc.gpsimd.collective_compute(
                    "ReduceScatter", OP.add,
                    replica_groups=[list(range(NC))],
                    ins=[rs_inB1.opt()],
                    outs=[rs_outB1.opt()],
                )
                for i in (6, 7):
                    last_dma = attn_tile(i, rs_inB2, (i - 6) * 128,
                                         after=last_dma)
                nc.gpsimd.collective_compute(
                    "ReduceScatter", OP.add,
                    replica_groups=[list(range(NC))],
                    ins=[rs_inB2.opt()],
                    outs=[rs_outB2.opt()],
                )

                # ---------------- epilogue for own 128 queries --------------
                fo = ph2w.tile([QPC, D + 1], F16, tag="fo", bufs=1)
                nc.sync.dma_start(fo[0:64, :], rs_outA[:])
                nc.sync.dma_start(fo[64:96, :], rs_outB1[:])
                nc.sync.dma_start(fo[96:128, :], rs_outB2[:])
                linv = stat.tile([128, 1], F32, tag="linv")
                nc.vector.reciprocal(linv[:], fo[:, D:D + 1])
                ao = scr_bf.tile([128, D], BF16, tag="tmb")
                nc.vector.tensor_scalar(ao[:], fo[:, 0:D], linv[:], None,
                                        OP.mult)
                aoT = scr_bf.tile([128, D], BF16, tag="tmb")
                aoT_v = aoT[:].rearrange("p (g x) -> p g x", g=2)
                for g in range(2):
                    transpose_to(
                        aoT_v[:, g:g + 1, :].rearrange("p g x -> p (g x)")
                        .rearrange("p (u x) -> p u x", u=4),
                        ao[:],
                        [g * 4 + u for u in range(4)],
                    )
                zps = mmps.tile([128, D], F32, tag="mm")
                for h in range(2):
                    for j in range(NJ):
                        nc.tensor.matmul(
                            zps[:, h * 512:(h + 1) * 512],
                            aoT[:, j * 128:(j + 1) * 128],
                            wo_l[j][:, h * 512:(h + 1) * 512],
                            start=(j == 0),
                            stop=(j == NJ - 1),
                        )
                hh = scr_f.tile([128, D], F32, tag="sqscr")
                nc.vector.tensor_add(hh[:], qs_f32[:], zps[:])
                orstd = rms_stats(hh[:])
                yv = scr_f.tile([128, D], F32, tag="sqscr")
                nc.vector.tensor_scalar(yv[:], hh[:], orstd[:], None, OP.mult)
                if apply_norm_weights:
                    nc.vector.tensor_tensor(yv[:], yv[:], won_sb[:],
                                            op=OP.mult)
                nc.sync.dma_start(out[:], yv[:])

    nc.compile()
    return nc


def _pe_table():
    half = D // 2
    inv_freq = np.exp(np.arange(half, dtype=np.float32)
                      * (-math.log(10000.0) / half))
    ang = np.arange(S, dtype=np.float32)[:, None] * inv_freq
    return np.concatenate([np.sin(ang), np.cos(ang)], axis=-1).astype(np.float32)


def _core_gidx(c):
    """Global query indices owned by core c: 64 batch-0 queries
    [64c, 64c+64), then batch-1 queries [32c, 32c+32) and
    [256+32c, 256+32c+32) — matching the RS_A / RS_B1 / RS_B2 splits."""
    return np.concatenate([
        np.arange(64 * c, 64 * c + 64),
        N + np.arange(32 * c, 32 * c + 32),
        N + 256 + np.arange(32 * c, 32 * c + 32),
    ])


def make_in_maps(np_inputs, apply_w=False):
    hid = np.asarray(np_inputs["hidden_states"], np.float32)
    pos = np.asarray(np_inputs["context_positions"])
    Wfc = np.ascontiguousarray(np.asarray(np_inputs["W_fc"], np.float32))
    Wq = np.ascontiguousarray(np.asarray(np_inputs["Wq"], np.float32))
    Wk = np.ascontiguousarray(np.asarray(np_inputs["Wk"], np.float32))
    Wv = np.ascontiguousarray(np.asarray(np_inputs["Wv"], np.float32))
    Wo = np.ascontiguousarray(np.asarray(np_inputs["Wo"], np.float32))

    x = hid.reshape(B, S, LD)
    p = np.clip(pos.astype(np.int64), 0, S - 1)
    p_flat = p.reshape(QT)
    PE = _pe_table()

    iota_np = np.tile(np.arange(CH, dtype=np.float32), (128, 1))
    ident_np = np.eye(128, dtype=np.float32).astype(ml_dtypes.bfloat16)
    identf_np = np.eye(128, dtype=np.float32)

    in_maps = []
    for c in range(NC):
        sl = slice(c * CH, (c + 1) * CH)
        xT_c = np.ascontiguousarray(
            np.concatenate([x[0, sl], x[1, sl]], axis=0).T)
        gidx = _core_gidx(c)
        xqT_a = np.ascontiguousarray(x[gidx // N, p_flat[gidx]].T)
        peq_a = np.ascontiguousarray(PE[p_flat[gidx]]).astype(ml_dtypes.bfloat16)
        pet_a = np.ascontiguousarray(PE[sl].T).astype(ml_dtypes.bfloat16)
        thr_a = np.ascontiguousarray(
            (p_flat.astype(np.float32) - c * CH).reshape(NC, 128).T)
        m = {
            "xT": xT_c, "xqT": xqT_a,
            "wfc": Wfc, "wq": Wq, "wk": Wk, "wv": Wv, "wo": Wo,
            "pet": pet_a, "peq": peq_a, "thr": thr_a,
            "iota": iota_np, "ident": ident_np, "identf": identf_np,
        }
        if apply_w:
            m["whn"] = np.tile(np.asarray(np_inputs["w_hidden_norm"], np.float32), (128, 1))
            m["wqn"] = np.tile(np.asarray(np_inputs["w_q_norm"], np.float32), (128, 1))
            m["wkn"] = np.tile(np.asarray(np_inputs["w_k_norm"], np.float32), (128, 1))
            m["won"] = np.tile(np.asarray(np_inputs["w_out_norm"], np.float32), (128, 1))
        in_maps.append(m)
    return in_maps


def assemble_out(results):
    y = np.zeros((QT, D), np.float32)
    for c in range(NC):
        y[_core_gidx(c)] = results[c]["out"]
    return y.reshape(B, N, D)


def kernel(**inputs) -> np.ndarray:
    w_h = np.asarray(inputs["w_hidden_norm"], np.float32)
    w_q = np.asarray(inputs["w_q_norm"], np.float32)
    w_k = np.asarray(inputs["w_k_norm"], np.float32)
    w_o = np.asarray(inputs["w_out_norm"], np.float32)
    apply_w = not (np.all(w_h == 1) and np.all(w_q == 1)
                   and np.all(w_k == 1) and np.all(w_o == 1))

    key = ("nc", apply_w)
    if key not in _CACHE:
        _CACHE[key] = _build(apply_w)
    nc = _CACHE[key]

    in_maps = make_in_maps(inputs, apply_w)

    trace = os.environ.get("KERNEL_TRACE", "0") == "1"
    if trace:
        try:
            import axon_prof
            axon_prof.install()
        except Exception:
            trace = False
    res = run_bass_kernel_spmd(nc, in_maps, list(range(NC)), trace=trace)
    global LAST_EXEC_NS
    LAST_EXEC_NS = res.exec_time_ns

    return assemble_out(res.results).astype(np.float32)


LAST_EXEC_NS = None
